# revision 74
# baseline (speedup 1.0000x reference)
"""Trainium2 Bass kernel for the RBF-SVM dual objective (nn_Model_51556787421664).

Computes: value = 0.5 * alpha^T G alpha - sum(alpha), where
  X = xs.reshape(N, T*D) @ W            [N=4096, F=2048]
  K_ij = exp(-gamma * ||X_i - X_j||^2),  gamma = 1/F
  G = (y y^T) * K  with y in {-1,+1}

Factorization used on device (exact, no d2 materialization):
  K_ij = a_i a_j exp(2*gamma*G_ij), a_i = exp(-gamma*||X_i||^2), G = X X^T
  alpha^T G alpha = sum_ij w_i w_j exp(2*gamma*(X X^T)_ij), w_i = y_i alpha_i a_i

v2: fp8(e4m3) + DoubleRow matmuls for both the encoder and the gram stage
(2x PE throughput vs bf16). X is quantized to fp8 once and used consistently
for the gram AND for sq_i = ||X_i||^2, so the diagonal K_ii == 1 cancellation
is exact. The gram contraction is split into two f-halves: the half-gram G0
(f 0..1023, gathered by AllGather#0 issued mid-encoder) is computed for all
8 row-blocks right after the encoder while AllGather#1 (f 1024..2047) is in
flight, then pass1 adds G1, exps, and reduces. This hides the second
collective behind ~45us of PE work.

Sharding: rows of X across 8 cores; each core computes a [512, 4096] block
of exp(2g*X X^T) and reduces against w on the PE; host sums 8 partial z
vectors and takes the final dot.
"""

import functools

import numpy as np
import ml_dtypes

try:
    import jax as _jax
    if not _jax.config.jax_compilation_cache_dir:
        _jax.config.update("jax_compilation_cache_dir", "/tmp/jaxcache")
        _jax.config.update("jax_persistent_cache_min_entry_size_bytes", -1)
        _jax.config.update("jax_persistent_cache_min_compile_time_secs", 0)
except Exception:
    pass

# --- problem constants (hardcoded per contract; kernel.py is self-contained) ---
N = 4096          # rows
KDIM = 8192       # T*D contraction
F = 2048          # feature dim
NCORES = 8
NLOC = N // NCORES          # 512 local rows
P = 128
KT = KDIM // P              # 64 k-subtiles
KP = KT // 2                # 32 k-pairs (DoubleRow)
FT = F // P                 # 16 f-tiles
# X^T is AllGathered in 2 halves of 8 f-tiles: half 0 fires mid-encoder
# (done before the encoder ends), half 1 fires at encoder end and hides
# behind gram pass 0. More granularity doesn't help: the CC stream is
# serial and small AllGathers run at lower bus efficiency.
NQ = 2
FQS = (10, 6)               # f-tiles per half: big pass0 covers the tail
                            # collective; small AG1 lands sooner
MT = NLOC // P              # 4 local row tiles
NB = N // NLOC              # 8 global row blocks
# K is symmetric and the output is a scalar, so core r computes only the
# column blocks c=(r+o)%8 for o=0..4: each off-diagonal block pair {r,c}
# is covered once (twice for the o=4 antipode pairs) and the HOST applies
# the per-block multiplier (2 for o=1..3, 1 for o=0 and o=4) in the final
# dot. No mirror reduction on device at all.
NBLK = NB // 2 + 1          # 5 owned column offsets per core
BLK_MULT = (1.0, 2.0, 2.0, 2.0, 1.0, 0.0, 0.0, 0.0)  # by offset (c-r)%8
GAMMA = 1.0 / F
WSCALE = 1024.0             # host prescale of W so fp8(W') stays normal

_FP8 = ml_dtypes.float8_e4m3   # bit-exact with TRN FP8_EXP4 (max 240, inf)


def _build_nc():
    import concourse.mybir as mybir
    import concourse.tile as tile
    from concourse import bacc

    FP8 = mybir.dt.float8e4
    FP32 = mybir.dt.float32
    FP16 = mybir.dt.float16
    BF16 = mybir.dt.bfloat16
    E_DT = mybir.dt.bfloat16     # exp(G) tiles / w vector: fp32r z-matmuls
                                 # lower to multi-pass fp32 HIGH (~430ns);
                                 # bf16 runs single-pass
    DR = mybir.MatmulPerfMode.DoubleRow
    ACT = mybir.ActivationFunctionType

    nc = bacc.Bacc("TRN2", target_bir_lowering=False, debug=False,
                   num_devices=NCORES)

    zt_d = nc.dram_tensor("zt", [P, KT, NLOC], FP8, kind="ExternalInput")
    w_d = nc.dram_tensor("wmat", [FT, P, KP, 2, P], FP8, kind="ExternalInput")
    beta_d = nc.dram_tensor("beta", [P, MT], FP32, kind="ExternalInput")
    z_out = nc.dram_tensor("z_out", [NB, NLOC], FP32, kind="ExternalOutput")
    w_out = nc.dram_tensor("w_out", [P, MT], FP32, kind="ExternalOutput")

    with tile.TileContext(nc) as tc:
        with (
            tc.tile_pool(name="persist", bufs=1) as persist,
            tc.tile_pool(name="dram", bufs=1, space="DRAM") as dram,
            tc.tile_pool(name="wstream", bufs=6) as wpool,
            tc.tile_pool(name="atmp", bufs=2) as atmp,
            tc.tile_pool(name="gath", bufs=8) as gpool,
            tc.tile_pool(name="epool", bufs=6) as epool,
            tc.tile_pool(name="ztmp", bufs=3) as ztmp,
            tc.tile_pool(name="apsum", bufs=2, space="PSUM") as apsum,
            tc.tile_pool(name="sqpsum", bufs=1, space="PSUM") as sqpsum,
            tc.tile_pool(name="gpsum", bufs=1, space="PSUM") as gpsum,
            tc.tile_pool(name="zpsum", bufs=1, space="PSUM") as zpsum,
        ):
            # ---- persistent SBUF ----
            # Z^T in 8 tiles of 8 k-subtiles each: ztg[g][:, u, :] is
            # k-subtile 8g+u. The first f's matmuls consume zt groups faster
            # than one queue can deliver 4MB, so the loads are spread over
            # all three queues, interleaved with the first W chunks below in
            # just-in-time order.
            ztg = [persist.tile([P, 8, NLOC], FP8, name=f"zt{g}")
                   for g in range(8)]

            def ztdma(eng, g):
                eng.dma_start(ztg[g][:], zt_d[:, 8 * g:8 * g + 8, :])

            # group 0 split so the very first matmul gates on 128KB only
            nc.gpsimd.dma_start(ztg[0][:, 0:2, :], zt_d[:, 0:2, :])
            nc.gpsimd.dma_start(ztg[0][:, 2:8, :], zt_d[:, 2:8, :])
            for g in (2, 4, 6):
                ztdma(nc.gpsimd, g)
            # W tiles f=0/f=1 emitted up front, chunked, with the odd zt
            # groups slotted between chunks — all writes land in emission
            # order BEFORE any matmul that reads them
            wts_pre = []
            for f0 in (0, 1):
                wt = wpool.tile([P, KP, 2, P], FP8, tag="w", name="wt")
                wq = nc.scalar if f0 == 0 else nc.sync
                chunks = ([(0, 2), (2, 8), (8, 16), (16, 24), (24, 32)]
                          if f0 == 0 else
                          [(0, 8), (8, 16), (16, 24), (24, 32)])
                ztins = ({(8, 16): 3, (24, 32): 7} if f0 == 0
                         else {(8, 16): 1, (24, 32): 5})
                for lo, hi in chunks:
                    wq.dma_start(wt[:, lo:hi, :, :], w_d[f0, :, lo:hi, :, :])
                    if (lo, hi) in ztins:
                        ztdma(wq, ztins[(lo, hi)])
                wts_pre.append(wt)
            beta_sb = persist.tile([P, MT], FP32, name="beta_sb")
            nc.sync.dma_start(beta_sb[:], beta_d[:])
            ones_sb = persist.tile([P, 1], BF16, name="ones_sb")
            nc.vector.memset(ones_sb[:], 1.0)
            # X^T fp8 in 4 quarter tiles (per-tile dep => each AG waits only
            # on its own quarter)
            xq = [persist.tile([P, FQS[q], NLOC], FP8, name=f"xq{q}")
                  for q in range(NQ)]
            s_acc = persist.tile([P, NLOC], FP32, name="s_acc")
            g0_sb = persist.tile([P, NBLK * MT, NLOC], FP16, name="g0_sb")
            w_sb = persist.tile([P, MT], E_DT, name="w_sb")

            # agin staged incrementally (one 64KB piece per f-tile, on the
            # scalar queue — NOT gpsimd, whose AG0 trigger blocks that queue
            # until the collective completes), so the trigger-gating piece
            # is tiny and the collective fires right after the last copy
            # three segments: pass0's gather split 8+2 so the big piece
            # fires at f=7 and lands mid-encoder; the 2-tile and 6-tile
            # pieces are small. AGS[s] f-tiles per segment.
            AGS = (8, 2, 6)
            SEG0 = (0, 8, 10)   # first f-tile of each segment
            ag_in = [dram.tile([P, AGS[s], NLOC], FP8, name=f"agin{s}")
                     for s in range(3)]
            ag_out = {}

            def emit_ag(s):
                agout = dram.tile([NB, P, AGS[s], NLOC], FP8,
                                  addr_space="Shared", name=f"agout{s}")
                nc.gpsimd.collective_compute(
                    "AllGather",
                    mybir.AluOpType.bypass,
                    ins=[ag_in[s][:]],
                    outs=[agout[:]],
                    replica_groups=[list(range(NCORES))],
                )
                ag_out[s] = agout

            def seg_of(f):
                for s in (2, 1, 0):
                    if f >= SEG0[s]:
                        return s, f - SEG0[s]

            pid_sync = nc.sync.partition_id()
            pid_scal = nc.scalar.partition_id()
            pid_gps = nc.gpsimd.partition_id()
            pre_gt = {}

            # ---- stage A: encoder X^T = (W'^T Z^T)/1024, fp8 DoubleRow ----
            for f in range(FT):
                if f < 2:
                    wt = wts_pre[f]
                else:
                    wt = wpool.tile([P, KP, 2, P], FP8, tag="w", name="wt")
                    wq = nc.scalar if f % 2 == 0 else nc.sync
                    if f == 2:
                        for c in range(4):
                            wq.dma_start(wt[:, 8 * c:8 * c + 8, :, :],
                                         w_d[f, :, 8 * c:8 * c + 8, :, :])
                    else:
                        wq.dma_start(wt[:], w_d[f])
                xp = apsum.tile([P, NLOC], FP32, tag="xp", name="xp")
                for t in range(KP):
                    nc.tensor.matmul(
                        xp[:], wt[:, t],
                        ztg[t // 4][:, 2 * (t % 4):2 * (t % 4) + 2, :],
                        start=(t == 0), stop=(t == KP - 1), perf_mode=DR)
                q = 0 if f < FQS[0] else 1
                fi = f if q == 0 else f - FQS[0]
                nc.scalar.activation(xq[q][:, fi, :], xp[:], ACT.Copy,
                                     scale=1.0 / WSCALE)
                seg, sfi = seg_of(f)
                nc.scalar.dma_start(ag_in[seg][:, sfi, :], xq[q][:, fi, :])
                sqx = atmp.tile([P, NLOC], FP32, tag="sqx", name="sqx")
                nc.scalar.activation(sqx[:], xq[q][:, fi, :], ACT.Square)
                if f == 0:
                    nc.vector.tensor_copy(s_acc[:], sqx[:])
                else:
                    nc.vector.tensor_add(s_acc[:], s_acc[:], sqx[:])
                if sfi == AGS[seg] - 1:
                    emit_ag(seg)
                if f == 9:
                    # first gathered block's loads on the gpsimd queue: it
                    # is idle between the AG0b and AG1 triggers (~149-165us)
                    # while sync/scalar are still saturated by the W stream
                    gt01 = gpool.tile([P, FQS[0], NLOC], FP8, tag="gt0",
                                      name="gt")
                    for t in range(4):
                        nc.gpsimd.dma_start(
                            gt01[:, 2 * t:2 * t + 2, :],
                            ag_out[0][(pid_gps + 1) % NB][:, 2 * t:2 * t + 2, :])
                    nc.gpsimd.dma_start(gt01[:, 8:10, :],
                                        ag_out[1][(pid_gps + 1) % NB])
                    pre_gt[(0, 1)] = gt01

            def emit_gt_dma(half, o):
                # one gather load; o=1 (first consumed after the collective
                # lands) is chunked across both queues so its first f-pair
                # arrives fast. Kicks are emitted ONE BLOCK AHEAD, not all
                # up front: the matmuls' DMA-completion semaphore waits are
                # queue-cumulative, so batched kicks make the first block
                # gate on the LAST transfer.
                gt = gpool.tile([P, FQS[half], NLOC], FP8,
                                tag=f"gt{half}", name="gt")
                if half == 0:
                    if o == 1:
                        for t in range(4):   # seg0: 4 f-pairs, chunked
                            eng, pid = ((nc.sync, pid_sync) if t % 2 == 0
                                        else (nc.scalar, pid_scal))
                            eng.dma_start(
                                gt[:, 2 * t:2 * t + 2, :],
                                ag_out[0][(pid + 1) % NB][:, 2 * t:2 * t + 2, :])
                        nc.sync.dma_start(
                            gt[:, 8:10, :], ag_out[1][(pid_sync + 1) % NB])
                    else:
                        eng, pid = ((nc.sync, pid_sync) if o % 2 == 1
                                    else (nc.scalar, pid_scal))
                        eng.dma_start(gt[:, 0:8, :],
                                      ag_out[0][(pid + o) % NB])
                        eng.dma_start(gt[:, 8:10, :],
                                      ag_out[1][(pid + o) % NB])
                else:
                    if o == 1:
                        for t in range(3):
                            eng, pid = ((nc.sync, pid_sync) if t % 2 == 0
                                        else (nc.scalar, pid_scal))
                            eng.dma_start(
                                gt[:, 2 * t:2 * t + 2, :],
                                ag_out[2][(pid + 1) % NB][:, 2 * t:2 * t + 2, :])
                    else:
                        eng, pid = ((nc.sync, pid_sync) if o % 2 == 1
                                    else (nc.scalar, pid_scal))
                        eng.dma_start(gt[:], ag_out[2][(pid + o) % NB])
                return gt

            def emit_block(half, o, src):
                npair = FQS[half] // 2
                gps = [gpsum.tile([P, NLOC], FP32, tag=f"g{m}",
                                  name=f"g{m}") for m in range(MT)]
                # o==1 runs t-major so chunked gt arrivals pipeline; others
                # run m-major so each psum tile's add/exp/z chain starts
                # while the remaining matmuls run
                order = ([(m, t) for t in range(npair) for m in range(MT)]
                         if o == 1 else
                         [(m, t) for m in range(MT) for t in range(npair)])
                for m, t in order:
                    nc.tensor.matmul(
                        gps[m][:],
                        xq[half][:, 2 * t:2 * t + 2, m * P:(m + 1) * P],
                        src[:, 2 * t:2 * t + 2, :],
                        start=(t == 0), stop=(t == npair - 1),
                        perf_mode=DR)
                if half == 0:
                    for m in range(MT):
                        nc.vector.tensor_copy(g0_sb[:, o * MT + m, :],
                                              gps[m][:])
                else:
                    zp = zpsum.tile([1, NLOC], FP32, tag="z", name="zp")
                    for m in range(MT):
                        nc.vector.tensor_add(gps[m][:], gps[m][:],
                                             g0_sb[:, o * MT + m, :])
                        et = epool.tile([P, NLOC], E_DT, tag="e", name="et")
                        nc.scalar.activation(et[:], gps[m][:], ACT.Exp,
                                             scale=2.0 * GAMMA)
                        nc.tensor.matmul(zp[:], w_sb[:, m:m + 1], et[:],
                                         start=(m == 0), stop=(m == MT - 1))
                    zs = ztmp.tile([1, NLOC], FP32, tag="zs", name="zs")
                    nc.vector.tensor_copy(zs[:], zp[:])
                    nc.sync.dma_start(z_out[(pid_sync + o) % NB], zs[:])

            # sq row-sums via bf16 ones-matmul (partition reduction), then w
            s16 = atmp.tile([P, NLOC], BF16, tag="s16", name="s16")
            nc.vector.tensor_copy(s16[:], s_acc[:])
            sqp = sqpsum.tile([P, MT], FP32, tag="sp", name="sp")
            for m in range(MT):
                nc.tensor.matmul(sqp[:, m:m + 1],
                                 s16[:, m * P:(m + 1) * P], ones_sb[:, 0:1],
                                 start=True, stop=True)
            a_sb = atmp.tile([P, MT], FP32, tag="a", name="a_sb")
            nc.scalar.activation(a_sb[:], sqp[:], ACT.Exp, scale=-GAMMA)
            nc.vector.tensor_mul(w_sb[:], a_sb[:], beta_sb[:])
            wf32 = atmp.tile([P, MT], FP32, tag="wf32", name="wf32")
            nc.vector.tensor_mul(wf32[:], a_sb[:], beta_sb[:])
            nc.sync.dma_start(w_out[:], wf32[:])
            # zero the three column-blocks this core does not own
            zzero = ztmp.tile([1, NLOC], FP32, tag="zz", name="zzero")
            nc.vector.memset(zzero[:], 0.0)
            for o in range(NBLK, NB):
                nc.sync.dma_start(z_out[(pid_sync + o) % NB], zzero[:])

            # the self block needs no gathered data: compute its full
            # 16-f-tile gram in one psum accumulation and exp straight from
            # psum (no fp16 staging, no DVE add on the chain) — m-major so
            # each exp/z chain hides behind the remaining matmuls
            gps = [gpsum.tile([P, NLOC], FP32, tag=f"g{m}", name=f"g{m}")
                   for m in range(MT)]
            pairs = ([(0, t) for t in range(FQS[0] // 2)]
                     + [(1, t) for t in range(FQS[1] // 2)])
            for m in range(MT):
                for i_p, (q, t) in enumerate(pairs):
                    nc.tensor.matmul(
                        gps[m][:],
                        xq[q][:, 2 * t:2 * t + 2, m * P:(m + 1) * P],
                        xq[q][:, 2 * t:2 * t + 2, :],
                        start=(i_p == 0), stop=(i_p == len(pairs) - 1),
                        perf_mode=DR)
            zp = zpsum.tile([1, NLOC], FP32, tag="z", name="zp")
            for m in range(MT):
                et = epool.tile([P, NLOC], E_DT, tag="e", name="et")
                nc.scalar.activation(et[:], gps[m][:], ACT.Exp,
                                     scale=2.0 * GAMMA)
                nc.tensor.matmul(zp[:], w_sb[:, m:m + 1], et[:],
                                 start=(m == 0), stop=(m == MT - 1))
            zs = ztmp.tile([1, NLOC], FP32, tag="zs", name="zs")
            nc.vector.tensor_copy(zs[:], zp[:])
            nc.sync.dma_start(z_out[pid_sync % NB], zs[:])

            # ---- gathered gram blocks in two f-halves, gather kicks one
            # block ahead ----
            nxt = {(0, 1): pre_gt[(0, 1)]}
            for half in (0, 1):
                for o in range(1, NBLK):
                    gt = nxt.pop((half, o))
                    if o + 1 < NBLK:
                        nxt[(half, o + 1)] = emit_gt_dma(half, o + 1)
                    elif half == 0:
                        nxt[(1, 1)] = emit_gt_dma(1, 1)
                    emit_block(half, o, gt)

    nc.compile()
    return nc


@functools.cache
def _get_nc():
    return _build_nc()


def _prep_in_maps(xs, W, ys, alphas):
    xs = np.asarray(xs, dtype=np.float32)
    W = np.asarray(W, dtype=np.float32)
    ys = np.asarray(ys)
    alphas = np.asarray(alphas, dtype=np.float32)

    beta = ((2 * ys - 1).astype(np.float32) * alphas)  # [N]
    X8 = np.ascontiguousarray(xs.reshape(N, KDIM)).astype(_FP8)  # [N, KDIM]
    W8 = (W * WSCALE).astype(_FP8)                               # [KDIM, F]
    # wmat [FT, P, KP, 2, P]: (t,ko,p,f,fc) <- row (2t+ko)*128+p, col f*128+fc
    w_t = np.ascontiguousarray(
        W8.reshape(KP, 2, P, FT, P).transpose(3, 2, 0, 1, 4))

    in_maps = []
    for c in range(NCORES):
        sl = slice(c * NLOC, (c + 1) * NLOC)
        # zt [P, KT, NLOC]: zt[p, m, j] = X8[row j, k=m*128+p]
        zt_c = np.ascontiguousarray(
            X8[sl].T.reshape(KT, P, NLOC).transpose(1, 0, 2))
        beta_c = np.ascontiguousarray(beta[sl].reshape(MT, P).T)  # [P, MT]
        in_maps.append({"zt": zt_c, "wmat": w_t, "beta": beta_c})
    return in_maps, alphas


def _finish(results, alphas):
    w_full = np.zeros(N, dtype=np.float64)
    for c, r in enumerate(results):
        sl = slice(c * NLOC, (c + 1) * NLOC)
        w_full[sl] = r["w_out"].astype(np.float64).T.reshape(NLOC)
    # core r's z_out row c holds u = w_r^T E_{r,c}; the block's contribution
    # to the quadratic form is mult * (u . w_c), mult keyed by (c-r)%8
    quad = 0.0
    for r_idx, r in enumerate(results):
        z = r["z_out"].astype(np.float64)
        for c in range(NB):
            m = BLK_MULT[(c - r_idx) % NB]
            if m:
                quad += m * float(
                    np.dot(z[c], w_full[c * NLOC:(c + 1) * NLOC]))
    value = 0.5 * quad - float(np.sum(alphas.astype(np.float64)))
    return np.array([[value]], dtype=np.float32)


class Runner:
    """Compiles once; keeps inputs on device for repeated timed execs."""

    def __init__(self):
        self.nc = _get_nc()
        self._jitted = None

    def run_spmd(self, in_maps):
        from concourse import bass_utils
        res = bass_utils.run_bass_kernel_spmd(
            self.nc, in_maps, core_ids=list(range(NCORES)))
        return res.results

    # -- custom PJRT path mirroring bass2jax.run_bass_via_pjrt, but keeping
    #    the jitted fn + device inputs so repeated executions can be timed --
    def prepare(self, in_maps):
        import jax
        import numpy as np
        from jax.sharding import Mesh, PartitionSpec
        from jax.experimental.shard_map import shard_map
        import concourse.mybir as mybir
        from concourse import bass2jax

        nc = self.nc
        bass2jax.install_neuronx_cc_hook()
        partition_name = (nc.partition_id_tensor.name
                          if nc.partition_id_tensor else None)
        in_names, out_names, out_avals, zero_outs = [], [], [], []
        for alloc in nc.m.functions[0].allocations:
            if not isinstance(alloc, mybir.MemoryLocationSet):
                continue
            name = alloc.memorylocations[0].name
            if alloc.kind == "ExternalInput":
                if name != partition_name:
                    in_names.append(name)
            elif alloc.kind == "ExternalOutput":
                out_names.append(name)
                shape = tuple(alloc.tensor_shape)
                dtype = mybir.dt.np(alloc.dtype)
                out_avals.append(jax.core.ShapedArray(shape, dtype))
                zero_outs.append(np.zeros(shape, dtype))
        n_params = len(in_names)
        n_outs = len(out_avals)
        all_names = in_names + out_names
        if partition_name is not None:
            all_names = all_names + [partition_name]

        def _body(*args):
            operands = list(args)
            if partition_name is not None:
                operands.append(bass2jax.partition_id_tensor())
            outs = bass2jax._bass_exec_p.bind(
                *operands,
                out_avals=tuple(out_avals),
                in_names=tuple(all_names),
                out_names=tuple(out_names),
                lowering_input_output_aliases=(),
                sim_require_finite=True,
                sim_require_nnan=True,
                nc=nc,
            )
            return tuple(outs)

        devices = jax.devices()[:NCORES]
        mesh = Mesh(np.asarray(devices), ("core",))
        in_specs = (PartitionSpec("core"),) * (n_params + n_outs)
        out_specs = (PartitionSpec("core"),) * n_outs
        donate = tuple(range(n_params, n_params + n_outs))
        self._jitted = jax.jit(
            shard_map(_body, mesh=mesh, in_specs=in_specs,
                      out_specs=out_specs, check_rep=False),
            donate_argnums=donate, keep_unused=True)
        concat_in = [
            np.concatenate([np.asarray(in_maps[c][nm]) for c in range(NCORES)],
                           axis=0)
            for nm in in_names
        ]
        self._sharding = jax.sharding.NamedSharding(mesh, PartitionSpec("core"))
        self._dev_in = [jax.device_put(a, self._sharding) for a in concat_in]
        self._zero_outs = zero_outs
        self._out_names = out_names
        self._out_avals = out_avals

    def _zeros_dev(self):
        import jax
        return [jax.device_put(
                    np.zeros((NCORES * z.shape[0], *z.shape[1:]), z.dtype),
                    self._sharding)
                for z in self._zero_outs]

    def exec_once(self):
        out_arrs = self._jitted(*self._dev_in, *self._zeros_dev())
        import jax
        jax.block_until_ready(out_arrs)
        return [
            {nm: np.asarray(out_arrs[i]).reshape(NCORES, *self._out_avals[i].shape)[c]
             for i, nm in enumerate(self._out_names)}
            for c in range(NCORES)
        ]

    def time(self, reps=10):
        import time
        self.exec_once()  # warm
        ts = []
        for _ in range(reps):
            zo = self._zeros_dev()
            import jax
            jax.block_until_ready(zo)
            t0 = time.perf_counter()
            out = self._jitted(*self._dev_in, *zo)
            jax.block_until_ready(out)
            ts.append(time.perf_counter() - t0)
        return min(ts), sorted(ts)[len(ts) // 2]


def kernel(**inputs) -> np.ndarray:
    in_maps, alphas = _prep_in_maps(**inputs)
    r = Runner()
    results = r.run_spmd(in_maps)
    return _finish(results, alphas)


if __name__ == "__main__":
    rng = np.random.default_rng(0)
    xs = rng.standard_normal((N, 64, 128), dtype=np.float32)
    W = (rng.standard_normal((KDIM, F), dtype=np.float32) / np.sqrt(KDIM)).astype(np.float32)
    ys = rng.integers(0, 2, N).astype(np.int32)
    alphas = rng.standard_normal(N, dtype=np.float32)
    out = kernel(xs=xs, W=W, ys=ys, alphas=alphas)
    print("kernel out:", out)
